# revision 1
# baseline (speedup 1.0000x reference)
"""DeformableConv1d Trainium2 kernel.

Problem: N=16, C_in=64, L=8192, K=3, C_out=64, PAD=1.
Sharding: data-parallel over batch; each of 8 cores handles 2 samples.

Math (validated against the jax reference):
  offsets = conv1d(x, w_off, pad=1) + b_off      (only channels 0,2,4 used)
  grid[l',k] = clip(l'+1 + off_k[l'], 0, 8193)   (padded coords)
  left = floor(grid), alpha = grid - left
  out[n, o, q*64+r] = sum_{k,t} w[o, k*64+t] * xd[n, r, t*128+q, k]
  xd[n, c, l', k] = (1-a)*xp[n, c, left] + a*xp[n, c, left+1]
where l = q*64+r (q in [0,128), r in [0,64)), t in [0,64).

Kernel structure per core:
  P1: load x -> SBUF; offsets conv on PE; PE-transpose x -> XT2[2*8195, 64]
      fp32 DRAM (row = padded position, all 64 channels); compute
      idx/alpha tiles [128,128] (partition p = 64n+t, free q).
  P2: for each output quarter Q (2048 cols) and tap k: indirect-DMA gather
      of row pairs (left, left+1) -> GB[128, 32, 128]; DVE blend
      D = R-L, E = alpha*D; PE matmuls acc out += WkT @ L + WkT @ E into
      PSUM [64, 2048] per sample; ACT drain (+bias); DMA out.
"""

import numpy as np

N, C, L, K, PAD = 16, 64, 8192, 3, 1
NS = 2                 # samples per core
NCORES = 8
LP = L + 2 * PAD       # 8194
XROWS = LP + 1         # 8195 rows per sample (incl. overflow row for left+1)
CO = 64
NQ = 4                 # output quarters
QW = L // NQ           # 2048 output cols per quarter
QB = QW // 64          # 32 q-values per quarter

_CACHE = {}


def _build_nc(debug=False, reps=1, stage='full'):
    import concourse.bass as bass
    import concourse.tile as tile
    from concourse import bacc, mybir
    from concourse.bass import IndirectOffsetOnAxis

    f32 = mybir.dt.float32
    i32 = mybir.dt.int32
    i16 = mybir.dt.int16
    Alu = mybir.AluOpType
    Act = mybir.ActivationFunctionType

    nc = bacc.Bacc("TRN2", target_bir_lowering=False)

    xin = nc.dram_tensor("xin", [NS, C, L], f32, kind="ExternalInput")
    woffT = nc.dram_tensor("woffT", [C, 9], f32, kind="ExternalInput")
    wTk2 = nc.dram_tensor("wTk2", [128, 192], f32, kind="ExternalInput")
    boff = nc.dram_tensor("boff", [3, 1], f32, kind="ExternalInput")
    bout = nc.dram_tensor("bout", [CO, 1], f32, kind="ExternalInput")
    base128 = nc.dram_tensor("base128", [128, 128], f32, kind="ExternalInput")
    row128 = nc.dram_tensor("row128", [128, 128], f32, kind="ExternalInput")
    ident = nc.dram_tensor("ident", [64, 64], f32, kind="ExternalInput")
    out = nc.dram_tensor("out", [NS, CO, L], f32, kind="ExternalOutput")

    if debug:
        xt2 = nc.dram_tensor("xt2", [NS * XROWS, C], f32, kind="ExternalOutput")
        d_offs = nc.dram_tensor("d_offs", [NS, 3, L], f32, kind="ExternalOutput")
        d_idx = nc.dram_tensor("d_idx", [K, 128, 128], i16, kind="ExternalOutput")
        d_alpha = nc.dram_tensor("d_alpha", [K, 128, 128], f32, kind="ExternalOutput")
        d_gb = nc.dram_tensor("d_gb", [128, QB, 128], f32, kind="ExternalOutput")
    else:
        xt2 = nc.dram_tensor("xt2", [NS * XROWS, C], f32)  # internal scratch
    idxd = nc.dram_tensor("idxd", [K, 16, 1024], mybir.dt.int16)  # wrapped idx bounce

    with tile.TileContext(nc) as tc:
      for rep in range(reps):
        with tc.tile_pool(name=f"const{rep}", bufs=1) as constp:
            woffT_t = constp.tile([C, 9], f32)
            nc.sync.dma_start(woffT_t[:], woffT[:])
            wTk2_t = constp.tile([128, 192], f32)
            nc.sync.dma_start(wTk2_t[:], wTk2[:])
            boff_t = constp.tile([3, 1], f32)
            nc.sync.dma_start(boff_t[:], boff[:])
            bout_t = constp.tile([CO, 1], f32)
            nc.sync.dma_start(bout_t[:], bout[:])
            base_t = constp.tile([128, 128], f32)
            nc.sync.dma_start(base_t[:], base128[:])
            row_t = constp.tile([128, 128], f32)
            nc.sync.dma_start(row_t[:], row128[:])
            id_t = constp.tile([64, 64], f32)
            nc.sync.dma_start(id_t[:], ident[:])
            zrow = constp.tile([1, C], f32)
            nc.vector.memset(zrow[:], 0.0)
            # zero pad rows of xt2 (rows 0, 8193, 8194 per sample)
            for n in range(NS):
                for r in (0, LP - 1, LP):
                    nc.sync.dma_start(xt2[n * XROWS + r : n * XROWS + r + 1, :],
                                      zrow[:1, :])

            alpha_t = [constp.tile([128, 128], f32, tag=f"alpha{k}", name=f"al{rep}_{k}") for k in range(K)]
            idx_t = [constp.tile([128, 128], i16, tag=f"idx{k}", name=f"ix{rep}_{k}") for k in range(K)]
            # wrapped+replicated gather index tiles for dma_gather (one per (k, Q)):
            # wkq[k][Q][16g+a, ql*8 + 4n + h] = XROWS*n + left_k[(16h+a)*128 + Q*32 + ql]
            wkq_t = [[constp.tile([128, 8 * QB], i16, tag=f"wk{k}_{Q}", name=f"wk{rep}_{k}_{Q}")
                      for Q in range(NQ)] for k in range(K)]

            # ---------------- phase 1 ----------------
            with tc.tile_pool(name=f"xp{rep}", bufs=2) as xpp, \
                 tc.tile_pool(name=f"offs{rep}", bufs=2) as offsp, \
                 tc.tile_pool(name=f"cpsum{rep}", bufs=1, space="PSUM") as cpsump, \
                 tc.tile_pool(name=f"tpsum{rep}", bufs=3, space="PSUM") as tpsump, \
                 tc.tile_pool(name=f"stage{rep}", bufs=3) as stagep, \
                 tc.tile_pool(name=f"small{rep}", bufs=2) as smallp:

                offs_tiles = []
                for n in range(NS):
                    xp = xpp.tile([C, LP], f32, tag="xp")
                    nc.vector.memset(xp[:, 0:1], 0.0)
                    nc.vector.memset(xp[:, LP - 1 : LP], 0.0)
                    nc.sync.dma_start(xp[:, 1 : 1 + L], xin[n])

                    # offsets conv: off[k, l'] = sum_c sum_j woff[k,c,j]*xp[c, l'+j]
                    offs_n = offsp.tile([3, L], f32, tag="offs")
                    offs_tiles.append(offs_n)
                    for c2 in range(L // 2048):
                        cps = cpsump.tile([3, 2048], f32, tag="cps")
                        for b in range(4):
                            col0 = c2 * 2048 + b * 512
                            for j in range(3):
                                nc.tensor.matmul(
                                    cps[:, b * 512 : (b + 1) * 512],
                                    lhsT=woffT_t[:, j * 3 : (j + 1) * 3],
                                    rhs=xp[:, j + col0 : j + col0 + 512],
                                    start=(j == 0), stop=(j == 2),
                                )
                        nc.scalar.activation(offs_n[:, c2 * 2048 : (c2 + 1) * 2048],
                                             cps[:], Act.Identity, bias=boff_t[:])

                    # transpose x into xt2 rows (row l+1 <- x[:, l])
                    for g in range(8):
                        l0 = g * 1024
                        tps = tpsump.tile([128, 512], f32, tag="tps")
                        for j in range(8):
                            nc.tensor.transpose(
                                tps[:, j * 64 : (j + 1) * 64],
                                xp[:, 1 + l0 + j * 128 : 1 + l0 + (j + 1) * 128],
                                id_t[:],
                            )
                        st = stagep.tile([128, 512], f32, tag="st")
                        nc.vector.tensor_copy(st[:], tps[:])
                        nc.sync.dma_start(
                            xt2[n * XROWS + 1 + l0 : n * XROWS + 1 + l0 + 1024, :]
                            .rearrange("(j p) c -> p j c", p=128),
                            st[:].rearrange("p (j c) -> p j c", c=64),
                        )

                # idx / alpha per tap k, in [p=64n+t, q] layout
                for k in range(K):
                    off128 = smallp.tile([128, 128], f32, tag="off128")
                    for n in range(NS):
                        nc.sync.dma_start(off128[n * 64 : (n + 1) * 64, :],
                                          offs_tiles[n][k : k + 1, :])
                    grid = smallp.tile([128, 128], f32, tag="grid")
                    nc.vector.tensor_tensor(grid[:], off128[:], base_t[:], op=Alu.add)
                    gridc = smallp.tile([128, 128], f32, tag="gridc")
                    nc.vector.tensor_scalar(gridc[:], grid[:], 0.0, float(LP - 1),
                                            op0=Alu.max, op1=Alu.min)
                    # floor(gridc), robust to cast rounding mode:
                    # c = cast(gridc); floor = c - (c > gridc)
                    casti = smallp.tile([128, 128], i32, tag="casti")
                    nc.vector.tensor_copy(casti[:], gridc[:])
                    castf = smallp.tile([128, 128], f32, tag="castf")
                    nc.vector.tensor_copy(castf[:], casti[:])
                    over = smallp.tile([128, 128], f32, tag="over")
                    nc.vector.tensor_tensor(over[:], castf[:], gridc[:], op=Alu.is_gt)
                    leftf = smallp.tile([128, 128], f32, tag="leftf")
                    nc.vector.tensor_tensor(leftf[:], castf[:], over[:],
                                            op=Alu.subtract)
                    nc.vector.tensor_tensor(alpha_t[k][:], gridc[:], leftf[:],
                                            op=Alu.subtract)
                    idxf = smallp.tile([128, 128], f32, tag="idxf")
                    nc.vector.tensor_tensor(idxf[:], leftf[:], row_t[:], op=Alu.add)
                    nc.vector.tensor_copy(idx_t[k][:], idxf[:])
                    # bounce to DRAM in wrapped layout:
                    # idxd[k][a, Q*256 + ql*8 + 4n + h] = idx16[64n+16h+a, Q*32+ql]
                    for n in range(NS):
                        for h in range(4):
                            src = idx_t[k][n * 64 + 16 * h : n * 64 + 16 * h + 16, :]
                            dst = bass.AP(
                                tensor=idxd[:].tensor,
                                offset=k * 16 * 1024 + 4 * n + h,
                                ap=[[1024, 16], [256, NQ], [8, QB]])
                            nc.sync.dma_start(dst, src)
                    # read back with 8x partition replication
                    for Q in range(NQ):
                        src = bass.AP(
                            tensor=idxd[:].tensor,
                            offset=k * 16 * 1024 + Q * 8 * QB,
                            ap=[[0, 8], [1024, 16], [1, 8 * QB]])
                        nc.sync.dma_start(wkq_t[k][Q][:], src)
                    if debug:
                        nc.sync.dma_start(d_idx[k], idx_t[k][:])
                        nc.sync.dma_start(d_alpha[k], alpha_t[k][:])
                if debug:
                    for n in range(NS):
                        nc.sync.dma_start(d_offs[n], offs_tiles[n][:])

            # ---------------- phase 2 ----------------
            if stage == 'p1':
                continue
            with tc.tile_pool(name=f"gb{rep}", bufs=3) as gbp, \
                 tc.tile_pool(name=f"dd{rep}", bufs=2) as ddp, \
                 tc.tile_pool(name=f"ee{rep}", bufs=2) as eep, \
                 tc.tile_pool(name=f"outst{rep}", bufs=2) as outp, \
                 tc.tile_pool(name=f"mpsum{rep}", bufs=1, space="PSUM") as mpsump:
                for Q in range(NQ):
                    ps = [mpsump.tile([CO, QW], f32, tag=f"ps{n}", name=f"ps{rep}_{n}_{Q}") for n in range(NS)] \
                        if stage != 'nomm' else None
                    for k in range(K):
                        gt = gbp.tile([128, QB, 128], f32, tag="gb")
                        xt2_pair = bass.AP(tensor=xt2[:].tensor, offset=0,
                                           ap=[[64, NS * XROWS - 1], [1, 128]])
                        nc.gpsimd.dma_gather(
                            gt[:], xt2_pair,
                            wkq_t[k][Q][:],
                            num_idxs=128 * QB, num_idxs_reg=128 * QB,
                            elem_size=128, elem_step=64, single_packet=False,
                        )
                        if debug and Q == 0 and k == 0:
                            nc.sync.dma_start(d_gb[:], gt[:])
                        if stage == 'nomm':
                            continue
                        dt_ = ddp.tile([128, QB, 64], f32, tag="dd")
                        nc.vector.tensor_tensor(dt_[:], gt[:, :, 64:128],
                                                gt[:, :, 0:64], op=Alu.subtract)
                        et = eep.tile([128, QB, 64], f32, tag="ee")
                        nc.vector.tensor_tensor(
                            et[:], dt_[:],
                            alpha_t[k][:, Q * QB : (Q + 1) * QB]
                            .to_broadcast([128, QB, 64]),
                            op=Alu.mult)
                        for n in range(NS):
                            lhs = wTk2_t[n * 64 : (n + 1) * 64, k * 64 : (k + 1) * 64]
                            for ridx, rhs_full in ((0, gt[n * 64 : (n + 1) * 64, :, 0:64]),
                                                   (1, et[n * 64 : (n + 1) * 64, :, :])):
                                for b in range(QW // 512):
                                    nc.tensor.matmul(
                                        ps[n][:, b * 512 : (b + 1) * 512],
                                        lhsT=lhs,
                                        rhs=rhs_full[:, b * 8 : (b + 1) * 8, :],
                                        start=(k == 0 and ridx == 0),
                                        stop=(k == K - 1 and ridx == 1),
                                    )
                    for n in range(NS if stage != 'nomm' else 0):
                        ot = outp.tile([CO, QW], f32, tag=f"ot{n}")
                        nc.scalar.activation(ot[:], ps[n][:], Act.Identity,
                                             bias=bout_t[:])
                        nc.sync.dma_start(out[n, :, Q * QW : (Q + 1) * QW], ot[:])

    nc.compile()
    return nc


def _host_tables(w_off, w, b_off, b):
    woffT = np.ascontiguousarray(
        w_off[[0, 2, 4], :, :].transpose(1, 2, 0).reshape(C, 9)).astype(np.float32)
    wTk = np.ascontiguousarray(
        w.reshape(CO, K, 64).transpose(2, 1, 0).reshape(64, K * CO)).astype(np.float32)
    wTk2 = np.concatenate([wTk, wTk], axis=0)
    p = np.arange(128)[:, None]
    q = np.arange(128)[None, :]
    base128 = ((p % 64) * 128 + q + 1).astype(np.float32)
    row128 = (XROWS * (p // 64) + 0 * q).astype(np.float32)
    boff3 = np.ascontiguousarray(b_off[[0, 2, 4]].reshape(3, 1)).astype(np.float32)
    bout = np.ascontiguousarray(b.reshape(CO, 1)).astype(np.float32)
    ident = np.eye(64, dtype=np.float32)
    return dict(woffT=woffT, wTk2=wTk2, base128=base128, row128=row128,
                boff=boff3, bout=bout, ident=ident)


def get_nc(debug=False, reps=1, stage='full'):
    key = f"nc_{int(debug)}_{reps}_{stage}"
    if key not in _CACHE:
        _CACHE[key] = _build_nc(debug, reps, stage)
    return _CACHE[key]


def _get_callable(debug=False, reps=1, stage='full'):
    """Jitted 8-core shard_map program running the NEFF; compiled once."""
    fkey = f"fn_{int(debug)}_{reps}_{stage}"
    if fkey in _CACHE:
        return _CACHE[fkey]
    import jax
    from jax.sharding import Mesh, PartitionSpec
    from jax.experimental.shard_map import shard_map
    from concourse import bass2jax, mybir

    bass2jax.install_neuronx_cc_hook()
    nc = get_nc(debug, reps, stage)
    partition_name = nc.partition_id_tensor.name if nc.partition_id_tensor else None
    in_names, out_names, out_avals = [], [], []
    for alloc in nc.m.functions[0].allocations:
        if not isinstance(alloc, mybir.MemoryLocationSet):
            continue
        name = alloc.memorylocations[0].name
        if alloc.kind == "ExternalInput":
            if name != partition_name:
                in_names.append(name)
        elif alloc.kind == "ExternalOutput":
            out_names.append(name)
            out_avals.append(jax.core.ShapedArray(
                tuple(alloc.tensor_shape), mybir.dt.np(alloc.dtype)))
    n_params = len(in_names)
    all_in_names = list(in_names) + list(out_names)
    if partition_name is not None:
        all_in_names.append(partition_name)

    def _body(*args):
        operands = list(args)
        if partition_name is not None:
            operands.append(bass2jax.partition_id_tensor())
        outs = bass2jax._bass_exec_p.bind(
            *operands,
            out_avals=tuple(out_avals),
            in_names=tuple(all_in_names),
            out_names=tuple(out_names),
            lowering_input_output_aliases=(),
            sim_require_finite=True,
            sim_require_nnan=True,
            nc=nc,
        )
        return tuple(outs)

    devices = jax.devices()[:NCORES]
    mesh = Mesh(np.asarray(devices), ("core",))
    n_all = n_params + len(out_names)
    sharded = jax.jit(
        shard_map(_body, mesh=mesh,
                  in_specs=(PartitionSpec("core"),) * n_all,
                  out_specs=(PartitionSpec("core"),) * len(out_names),
                  check_rep=False),
        keep_unused=True,
    )
    _CACHE[fkey] = (sharded, in_names, out_names, out_avals, mesh)
    return _CACHE[fkey]


def _concat_inputs(x, w_off, b_off, w, b, in_names, out_avals):
    tables = _host_tables(np.asarray(w_off), np.asarray(w),
                          np.asarray(b_off), np.asarray(b))
    x = np.ascontiguousarray(np.asarray(x), dtype=np.float32)
    per_core = []
    for i in range(NCORES):
        m = dict(tables)
        m["xin"] = np.ascontiguousarray(x[i * NS:(i + 1) * NS])
        per_core.append(m)
    concat = [np.concatenate([per_core[c][nm] for c in range(NCORES)], axis=0)
              for nm in in_names]
    zeros = [np.zeros((NCORES * av.shape[0], *av.shape[1:]), av.dtype)
             for av in out_avals]
    return concat + zeros


def kernel(x, w_off, b_off, w, b):
    fn, in_names, out_names, out_avals, mesh = _get_callable()
    args = _concat_inputs(x, w_off, b_off, w, b, in_names, out_avals)
    outs = fn(*args)
    oidx = out_names.index("out")
    full = np.asarray(outs[oidx]).reshape(NCORES * NS, CO, L).astype(np.float32)
    return full


def timeit(x, w_off, b_off, w, b, iters=30, reps=1, stage='full'):
    import time
    import jax
    from jax.sharding import NamedSharding, PartitionSpec
    fn, in_names, out_names, out_avals, mesh = _get_callable(reps=reps, stage=stage)
    args = _concat_inputs(x, w_off, b_off, w, b, in_names, out_avals)
    sh = NamedSharding(mesh, PartitionSpec("core"))
    dargs = [jax.device_put(a, sh) for a in args]
    outs = fn(*dargs)
    jax.block_until_ready(outs)
    t0 = time.perf_counter()
    for _ in range(iters):
        outs = fn(*dargs)
    jax.block_until_ready(outs)
    t1 = time.perf_counter()
    return (t1 - t0) / iters * 1e9



# revision 6
# speedup vs baseline: 1.1123x; 1.1123x over previous
"""DeformableConv1d Trainium2 kernel (bf16 gather rewrite).

Problem: N=16, C_in=64, L=8192, K=3, C_out=64, PAD=1.
Sharding: data-parallel over batch; each of 8 cores handles 2 samples.

Math (validated against the jax reference):
  offsets = conv1d(x, w_off, pad=1) + b_off      (only channels 0,2,4 used)
  grid[l',k] = clip(l'+1 + off_k[l'], 0, 8193)   (padded coords)
  left = floor(grid), alpha = grid - left
  out[n, o, q*64+r] = sum_{k,t} w[o, k*64+t] * xd[n, r, t*128+q, k]
  xd[n, c, l', k] = (1-a)*xp[n, c, left] + a*xp[n, c, left+1]
where l = q*64+r (q in [0,128), r in [0,64)), t in [0,64).

Kernel structure per core (all hot-path data in bf16):
  P1: load x (f32) -> convert bf16 xb; offsets conv on PE (bf16 in, f32 psum);
      PE-transpose xb -> xt2p[2*8194, 128] bf16 DRAM pair-token table
      (token l' = [x_pad[:,l'] | x_pad[:,l'+1]], 256B); idx/alpha tiles
      [128,128] (partition p = 64n+t, free q); wrapped gather-index bounce.
  P2: for each quarter Q and tap k: SWDGE dma_gather (256B tokens,
      round-robin over 4 queues) -> GB[128, 32, 128] bf16; DVE blend
      xd = L + alpha*(R-L); PE matmuls (bf16) acc out += WkT @ xd into
      PSUM [64, 2048] per sample; ACT drain (+bias); DMA out.
"""

import numpy as np

N, C, L, K, PAD = 16, 64, 8192, 3, 1
NS = 2                 # samples per core
NCORES = 8
LP = L + 2 * PAD       # 8194
RR = LP                # tokens per sample (l' in [0, 8193])
CO = 64
NQ = 4                 # output quarters
QW = L // NQ           # 2048 output cols per quarter
QB = QW // 64          # 32 q-values per quarter
NQUEUES = 4            # SWDGE gather queues

_CACHE = {}


def _build_nc(debug=False, reps=1, stage='full'):
    import concourse.bass as bass
    import concourse.tile as tile
    from concourse import bacc, mybir

    f32 = mybir.dt.float32
    bf16 = mybir.dt.bfloat16
    i32 = mybir.dt.int32
    i16 = mybir.dt.int16
    Alu = mybir.AluOpType
    Act = mybir.ActivationFunctionType

    nc = bacc.Bacc("TRN2", target_bir_lowering=False, num_swdge_queues=NQUEUES)

    xin = nc.dram_tensor("xin", [NS, C, L], f32, kind="ExternalInput")
    # wb: bf16 [128, 9+192]: rows 0:64 cols 0:9 = woffT [c, j*3+kk];
    # cols 9:201 = wTk [t, k*64+o] duplicated in both partition halves
    # (PE matmul needs lhsT base partition == rhs base partition).
    wb = nc.dram_tensor("wb", [128, 9 + K * CO], bf16, kind="ExternalInput")
    # bias: f32 [64, 2]: col 0 = bout (64 rows); col 1 rows 0:3 = boff
    biasd = nc.dram_tensor("biasd", [CO, 2], f32, kind="ExternalInput")
    out = nc.dram_tensor("out", [NS, CO, L], f32, kind="ExternalOutput")

    if debug:
        xt2p = nc.dram_tensor("xt2p", [NS * RR, 128], bf16, kind="ExternalOutput")
        d_offs = nc.dram_tensor("d_offs", [NS, 3, L], f32, kind="ExternalOutput")
        d_idx = nc.dram_tensor("d_idx", [K, 128, 128], i16, kind="ExternalOutput")
        d_alpha = nc.dram_tensor("d_alpha", [K, 128, 128], bf16, kind="ExternalOutput")
        d_gb = nc.dram_tensor("d_gb", [128, QB, 128], bf16, kind="ExternalOutput")
    else:
        xt2p = nc.dram_tensor("xt2p", [NS * RR, 128], bf16)  # internal scratch
    idxd = nc.dram_tensor("idxd", [K, 16, 1024], i16)  # wrapped idx bounce

    with tile.TileContext(nc) as tc:
      for rep in range(reps):
        with tc.tile_pool(name=f"const{rep}", bufs=1) as constp:
            wb_t = constp.tile([128, 9 + K * CO], bf16)
            nc.sync.dma_start(wb_t[:], wb[:])
            bias_t = constp.tile([CO, 2], f32)
            nc.sync.dma_start(bias_t[:], biasd[:])

            # --- device-generated constants ---
            # identity [64, 64] bf16 for PE transposes
            qi = constp.tile([64, 64], i32)
            nc.gpsimd.iota(qi[:], [[1, 64]], base=0, channel_multiplier=0)
            pv = constp.tile([64, 1], i32)
            nc.gpsimd.iota(pv[:], [[1, 1]], base=0, channel_multiplier=1)
            qi_f = constp.tile([64, 64], f32)
            nc.vector.tensor_copy(qi_f[:], qi[:])
            pv_f = constp.tile([64, 1], f32)
            nc.vector.tensor_copy(pv_f[:], pv[:])
            idn = constp.tile([64, 64], bf16)
            nc.vector.tensor_scalar(idn[:], qi_f[:], pv_f[:], None, op0=Alu.is_equal)
            # base128[p, q] = (p % 64) * 128 + q + 1  (f32)
            bi = constp.tile([128, 128], i32)
            nc.gpsimd.iota(bi[:], [[1, 128]], base=1, channel_multiplier=128)
            base_t = constp.tile([128, 128], f32)
            nc.vector.tensor_copy(base_t[0:64, :], bi[0:64, :])
            nc.vector.tensor_scalar(base_t[64:128, :], bi[64:128, :], 64 * 128,
                                    None, op0=Alu.subtract)
            # rowc[p] = RR * (p // 64)  (f32 per-partition scalar)
            rowc = constp.tile([128, 1], f32)
            nc.vector.memset(rowc[0:64, :], 0.0)
            nc.vector.memset(rowc[64:128, :], float(RR))
            # zero row for xt2p boundary tokens
            zrow = constp.tile([1, 128], bf16)
            nc.vector.memset(zrow[:], 0.0)

            alpha_t = [constp.tile([128, 128], bf16, tag=f"alpha{k}",
                                   name=f"al{rep}_{k}") for k in range(K)]
            # wrapped+replicated gather index tiles, all quarters:
            # wkq[k][16g+a, Q*256 + ql*8 + 4n + h] = RR*n + left_k[(16h+a)*128 + Q*32 + ql]
            wkq_t = [constp.tile([128, NQ * 256], i16, tag=f"wk{k}",
                                 name=f"wk{rep}_{k}") for k in range(K)]

            # ---------------- phase 1 ----------------
            with tc.tile_pool(name=f"xp{rep}", bufs=2) as xpp, \
                 tc.tile_pool(name=f"xb{rep}", bufs=2) as xbp, \
                 tc.tile_pool(name=f"offs{rep}", bufs=2) as offsp, \
                 tc.tile_pool(name=f"cpsum{rep}", bufs=1, space="PSUM") as cpsump, \
                 tc.tile_pool(name=f"tpsum{rep}", bufs=3, space="PSUM") as tpsump, \
                 tc.tile_pool(name=f"stage{rep}", bufs=3) as stagep, \
                 tc.tile_pool(name=f"small{rep}", bufs=2) as smallp:

                offs_tiles = []
                for n in range(NS):
                    xp = xpp.tile([C, L], f32, tag="xp")
                    nc.sync.dma_start(xp[:], xin[n])
                    xb = xbp.tile([C, LP], bf16, tag="xb")
                    nc.vector.memset(xb[:, 0:1], 0.0)
                    nc.vector.memset(xb[:, LP - 1 : LP], 0.0)
                    nc.scalar.activation(xb[:, 1 : 1 + L], xp[:], Act.Copy)

                    # offsets conv: off[kk, l'] = sum_c sum_j woff[kk,c,j]*xpad[c, l'+j]
                    offs_n = offsp.tile([3, L], f32, tag="offs")
                    offs_tiles.append(offs_n)
                    for c2 in range(L // 2048):
                        cps = cpsump.tile([3, 2048], f32, tag="cps")
                        for b in range(4):
                            col0 = c2 * 2048 + b * 512
                            for j in range(3):
                                nc.tensor.matmul(
                                    cps[:, b * 512 : (b + 1) * 512],
                                    lhsT=wb_t[0:64, j * 3 : (j + 1) * 3],
                                    rhs=xb[:, j + col0 : j + col0 + 512],
                                    start=(j == 0), stop=(j == 2),
                                )
                        nc.scalar.activation(offs_n[:, c2 * 2048 : (c2 + 1) * 2048],
                                             cps[:], Act.Identity,
                                             bias=bias_t[0:3, 1:2])

                    # transpose xb into xt2p pair-tokens
                    for g in range(8):
                        l0 = g * 1024
                        tps = tpsump.tile([128, 512], bf16, tag="tps")
                        for j in range(8):
                            nc.tensor.transpose(
                                tps[:, j * 64 : (j + 1) * 64],
                                xb[:, 1 + l0 + j * 128 : 1 + l0 + (j + 1) * 128],
                                idn[:],
                            )
                        st = stagep.tile([128, 512], bf16, tag="st")
                        nc.vector.tensor_copy(st[:], tps[:])
                        # left halves: token l' = l0+1 .. l0+1024
                        nc.sync.dma_start(
                            xt2p[n * RR + 1 + l0 : n * RR + 1 + l0 + 1024, 0:64]
                            .rearrange("(j p) c -> p j c", p=128),
                            st[:].rearrange("p (j c) -> p j c", c=64),
                        )
                        # right halves: token l' = l0 .. l0+1023
                        nc.sync.dma_start(
                            xt2p[n * RR + l0 : n * RR + l0 + 1024, 64:128]
                            .rearrange("(j p) c -> p j c", p=128),
                            st[:].rearrange("p (j c) -> p j c", c=64),
                        )
                    # boundary tokens: 0 left = xpad[0] = 0; 8192 right = xpad[8193] = 0;
                    # 8193 = [xpad[8193] | overflow] = 0
                    nc.sync.dma_start(xt2p[n * RR : n * RR + 1, 0:64], zrow[:1, 0:64])
                    nc.sync.dma_start(xt2p[n * RR + RR - 2 : n * RR + RR - 1, 64:128],
                                      zrow[:1, 0:64])
                    nc.sync.dma_start(xt2p[n * RR + RR - 1 : n * RR + RR, :],
                                      zrow[:1, :])

                # idx / alpha per tap k, in [p=64n+t, q] layout
                for k in range(K):
                    off128 = smallp.tile([128, 128], f32, tag="off128")
                    for n in range(NS):
                        nc.sync.dma_start(off128[n * 64 : (n + 1) * 64, :],
                                          offs_tiles[n][k : k + 1, :])
                    grid = smallp.tile([128, 128], f32, tag="grid")
                    nc.vector.tensor_tensor(grid[:], off128[:], base_t[:], op=Alu.add)
                    gridc = smallp.tile([128, 128], f32, tag="gridc")
                    nc.vector.tensor_scalar(gridc[:], grid[:], 0.0, float(LP - 1),
                                            op0=Alu.max, op1=Alu.min)
                    # floor(gridc), robust to cast rounding mode:
                    # c = cast(gridc); floor = c - (c > gridc)
                    casti = smallp.tile([128, 128], i32, tag="casti")
                    nc.vector.tensor_copy(casti[:], gridc[:])
                    castf = smallp.tile([128, 128], f32, tag="castf")
                    nc.vector.tensor_copy(castf[:], casti[:])
                    over = smallp.tile([128, 128], f32, tag="over")
                    nc.vector.tensor_tensor(over[:], castf[:], gridc[:], op=Alu.is_gt)
                    leftf = smallp.tile([128, 128], f32, tag="leftf")
                    nc.vector.tensor_tensor(leftf[:], castf[:], over[:],
                                            op=Alu.subtract)
                    alphaf = smallp.tile([128, 128], f32, tag="alphaf")
                    nc.vector.tensor_tensor(alphaf[:], gridc[:], leftf[:],
                                            op=Alu.subtract)
                    nc.vector.tensor_copy(alpha_t[k][:], alphaf[:])
                    idxf = smallp.tile([128, 128], f32, tag="idxf")
                    nc.vector.tensor_scalar(idxf[:], leftf[:], rowc[:, 0:1], None,
                                            op0=Alu.add)
                    idx16 = smallp.tile([128, 128], i16, tag="idx16")
                    nc.vector.tensor_copy(idx16[:], idxf[:])
                    # bounce to DRAM in wrapped layout (one write per (k, n)):
                    # idxd[k][a, 256Q + 8ql + 4n + h] = idx16[64n+16h+a, 32Q+ql]
                    for n in range(NS):
                        dst = bass.AP(
                            tensor=idxd[:].tensor,
                            offset=k * 16 * 1024 + 4 * n,
                            ap=[[1, 4], [1024, 16], [256, NQ], [8, QB]])
                        nc.sync.dma_start(dst, idx16[n * 64 : (n + 1) * 64, :])
                    # read back all quarters with 8x partition replication
                    src = bass.AP(
                        tensor=idxd[:].tensor,
                        offset=k * 16 * 1024,
                        ap=[[0, 8], [1024, 16], [1, 1024]])
                    nc.sync.dma_start(wkq_t[k][:], src)
                    if debug:
                        nc.sync.dma_start(d_idx[k], idx16[:])
                        nc.sync.dma_start(d_alpha[k], alpha_t[k][:])
                if debug:
                    for n in range(NS):
                        nc.sync.dma_start(d_offs[n], offs_tiles[n][:])

            # ---------------- phase 2 ----------------
            if stage == 'p1':
                continue
            gq = 0
            with tc.tile_pool(name=f"gb{rep}", bufs=4) as gbp, \
                 tc.tile_pool(name=f"dd{rep}", bufs=2) as ddp, \
                 tc.tile_pool(name=f"ee{rep}", bufs=2) as eep, \
                 tc.tile_pool(name=f"xd{rep}", bufs=2) as xdp, \
                 tc.tile_pool(name=f"outst{rep}", bufs=2) as outp, \
                 tc.tile_pool(name=f"mpsum{rep}", bufs=1, space="PSUM") as mpsump:
                for Q in range(NQ):
                    ps = [mpsump.tile([CO, QW], f32, tag=f"ps{n}",
                                      name=f"ps{rep}_{n}_{Q}") for n in range(NS)] \
                        if stage != 'nomm' else None
                    for k in range(K):
                        gt = gbp.tile([128, QB, 128], bf16, tag="gb")
                        xt2p_ap = bass.AP(tensor=xt2p[:].tensor, offset=0,
                                          ap=[[128, NS * RR], [1, 128]])
                        nc.gpsimd.dma_gather(
                            gt[:], xt2p_ap,
                            wkq_t[k][:, Q * 256 : (Q + 1) * 256],
                            num_idxs=128 * QB, num_idxs_reg=128 * QB,
                            elem_size=128, elem_step=128, single_packet=False,
                            queue_num=gq % NQUEUES,
                        )
                        gq += 1
                        if debug and Q == 0 and k == 0:
                            nc.sync.dma_start(d_gb[:], gt[:])
                        if stage == 'nomm':
                            continue
                        dt_ = ddp.tile([128, QB, 64], bf16, tag="dd")
                        nc.vector.tensor_tensor(dt_[:], gt[:, :, 64:128],
                                                gt[:, :, 0:64], op=Alu.subtract)
                        et = eep.tile([128, QB, 64], bf16, tag="ee")
                        nc.vector.tensor_tensor(
                            et[:], dt_[:],
                            alpha_t[k][:, Q * QB : (Q + 1) * QB]
                            .to_broadcast([128, QB, 64]),
                            op=Alu.mult)
                        xd = xdp.tile([128, QB, 64], bf16, tag="xd")
                        nc.vector.tensor_tensor(xd[:], gt[:, :, 0:64], et[:],
                                                op=Alu.add)
                        for n in range(NS):
                            lhs = wb_t[n * 64 : (n + 1) * 64,
                                       9 + k * 64 : 9 + (k + 1) * 64]
                            rhs_n = xd[n * 64 : (n + 1) * 64, :, :]
                            for b in range(QW // 512):
                                nc.tensor.matmul(
                                    ps[n][:, b * 512 : (b + 1) * 512],
                                    lhsT=lhs,
                                    rhs=rhs_n[:, b * 8 : (b + 1) * 8, :],
                                    start=(k == 0),
                                    stop=(k == K - 1),
                                )
                    for n in range(NS if stage != 'nomm' else 0):
                        ot = outp.tile([CO, QW], f32, tag=f"ot{n}")
                        nc.scalar.activation(ot[:], ps[n][:], Act.Identity,
                                             bias=bias_t[:, 0:1])
                        nc.sync.dma_start(out[n, :, Q * QW : (Q + 1) * QW], ot[:])

    nc.compile()
    return nc


def _host_tables(w_off, w, b_off, b):
    from concourse import mybir
    bf = mybir.dt.np(mybir.dt.bfloat16)
    woffT = np.ascontiguousarray(
        w_off[[0, 2, 4], :, :].transpose(1, 2, 0).reshape(C, 9)).astype(np.float32)
    wTk = np.ascontiguousarray(
        w.reshape(CO, K, 64).transpose(2, 1, 0).reshape(64, K * CO)).astype(np.float32)
    wb1 = np.concatenate([woffT, wTk], axis=1)
    wb2 = np.concatenate([np.zeros_like(woffT), wTk], axis=1)
    wb = np.concatenate([wb1, wb2], axis=0).astype(bf)
    bias = np.zeros((CO, 2), np.float32)
    bias[:, 0] = np.asarray(b, np.float32)
    bias[0:3, 1] = np.asarray(b_off, np.float32)[[0, 2, 4]]
    return dict(wb=wb, biasd=bias)


def get_nc(debug=False, reps=1, stage='full'):
    key = f"nc_{int(debug)}_{reps}_{stage}"
    if key not in _CACHE:
        _CACHE[key] = _build_nc(debug, reps, stage)
    return _CACHE[key]


def _get_callable(debug=False, reps=1, stage='full'):
    """Fast-dispatch 8-core shard_map program running the NEFF; compiled once."""
    fkey = f"fn_{int(debug)}_{reps}_{stage}"
    if fkey in _CACHE:
        return _CACHE[fkey]
    import jax
    from jax.sharding import Mesh, PartitionSpec, NamedSharding
    from jax.experimental.shard_map import shard_map
    from concourse import bass2jax, mybir

    bass2jax.install_neuronx_cc_hook()
    nc = get_nc(debug, reps, stage)
    partition_name = nc.partition_id_tensor.name if nc.partition_id_tensor else None
    in_names, out_names, out_avals = [], [], []
    for alloc in nc.m.functions[0].allocations:
        if not isinstance(alloc, mybir.MemoryLocationSet):
            continue
        name = alloc.memorylocations[0].name
        if alloc.kind == "ExternalInput":
            if name != partition_name:
                in_names.append(name)
        elif alloc.kind == "ExternalOutput":
            out_names.append(name)
            out_avals.append(jax.core.ShapedArray(
                tuple(alloc.tensor_shape), mybir.dt.np(alloc.dtype)))
    n_params = len(in_names)
    all_in_names = list(in_names) + list(out_names)
    if partition_name is not None:
        all_in_names.append(partition_name)

    def _body(*args):
        operands = list(args)
        if partition_name is not None:
            operands.append(bass2jax.partition_id_tensor())
        outs = bass2jax._bass_exec_p.bind(
            *operands,
            out_avals=tuple(out_avals),
            in_names=tuple(all_in_names),
            out_names=tuple(out_names),
            lowering_input_output_aliases=(),
            sim_require_finite=True,
            sim_require_nnan=True,
            nc=nc,
        )
        return tuple(outs)

    devices = jax.devices()[:NCORES]
    mesh = Mesh(np.asarray(devices), ("core",))
    n_all = n_params + len(out_names)
    jitted = jax.jit(
        shard_map(_body, mesh=mesh,
                  in_specs=(PartitionSpec("core"),) * n_all,
                  out_specs=(PartitionSpec("core"),) * len(out_names),
                  check_rep=False),
        keep_unused=True,
    )
    sh = NamedSharding(mesh, PartitionSpec("core"))
    # Compile the fast-dispatch executable against representative avals.
    dummies = []
    for nm in in_names:
        for alloc in nc.m.functions[0].allocations:
            if (isinstance(alloc, mybir.MemoryLocationSet)
                    and alloc.memorylocations[0].name == nm):
                shp = tuple(alloc.tensor_shape)
                dummies.append(jax.device_put(np.zeros(
                    (NCORES * shp[0], *shp[1:]), mybir.dt.np(alloc.dtype)), sh))
                break
    out_dummies = [jax.device_put(np.zeros(
        (NCORES * av.shape[0], *av.shape[1:]), av.dtype), sh)
        for av in out_avals]
    dummies += out_dummies
    fast = bass2jax.fast_dispatch_compile(lambda: jitted.lower(*dummies).compile())
    _CACHE[fkey] = (fast, in_names, out_names, out_avals, mesh, sh, out_dummies)
    return _CACHE[fkey]


def _device_inputs(x, w_off, b_off, w, b, in_names, sh):
    """Per-call device inputs. x is sliced by the sharding directly; small
    tables are tiled per-core."""
    import jax
    tables = _host_tables(np.asarray(w_off), np.asarray(w),
                          np.asarray(b_off), np.asarray(b))
    x = np.ascontiguousarray(np.asarray(x), dtype=np.float32)
    arrs = []
    for nm in in_names:
        if nm == "xin":
            arrs.append(x)
        else:
            t = tables[nm]
            arrs.append(np.tile(t, (NCORES,) + (1,) * (t.ndim - 1)))
    return [jax.device_put(a, sh) for a in arrs]


def kernel(x, w_off, b_off, w, b):
    fast, in_names, out_names, out_avals, mesh, sh, out_bufs = _get_callable()
    dargs = _device_inputs(x, w_off, b_off, w, b, in_names, sh) + out_bufs
    outs = fast(*dargs)
    oidx = out_names.index("out")
    return np.asarray(outs[oidx])


def timeit(x, w_off, b_off, w, b, iters=30, reps=1, stage='full'):
    import time
    import jax
    fast, in_names, out_names, out_avals, mesh, sh, out_bufs = \
        _get_callable(reps=reps, stage=stage)
    dargs = _device_inputs(x, w_off, b_off, w, b, in_names, sh) + out_bufs
    outs = fast(*dargs)
    jax.block_until_ready(outs)
    t0 = time.perf_counter()
    for _ in range(iters):
        outs = fast(*dargs)
    jax.block_until_ready(outs)
    t1 = time.perf_counter()
    return (t1 - t0) / iters * 1e9


# revision 10
# speedup vs baseline: 2.5420x; 2.2854x over previous
"""DeformableConv1d Trainium2 kernel (bf16 gather rewrite).

Problem: N=16, C_in=64, L=8192, K=3, C_out=64, PAD=1.
Sharding: data-parallel over batch; each of 8 cores handles 2 samples.

Math (validated against the jax reference):
  offsets = conv1d(x, w_off, pad=1) + b_off      (only channels 0,2,4 used)
  grid[l',k] = clip(l'+1 + off_k[l'], 0, 8193)   (padded coords)
  left = floor(grid), alpha = grid - left
  out[n, o, q*64+r] = sum_{k,t} w[o, k*64+t] * xd[n, r, t*128+q, k]
  xd[n, c, l', k] = (1-a)*xp[n, c, left] + a*xp[n, c, left+1]
where l = q*64+r (q in [0,128), r in [0,64)), t in [0,64).

Kernel structure per core (all hot-path data in bf16):
  P1: load x (f32) -> convert bf16 xb; offsets conv on PE (bf16 in, f32 psum);
      PE-transpose xb -> xt2p[2*8194, 128] bf16 DRAM pair-token table
      (token l' = [x_pad[:,l'] | x_pad[:,l'+1]], 256B); idx/alpha tiles
      [128,128] (partition p = 64n+t, free q); wrapped gather-index bounce.
  P2: for each quarter Q and tap k: SWDGE dma_gather (256B tokens,
      round-robin over 4 queues) -> GB[128, 32, 128] bf16; DVE blend
      xd = L + alpha*(R-L); PE matmuls (bf16) acc out += WkT @ xd into
      PSUM [64, 2048] per sample; ACT drain (+bias); DMA out.
"""

import numpy as np

N, C, L, K, PAD = 16, 64, 8192, 3, 1
NS = 2                 # samples per core
NCORES = 8
LP = L + 2 * PAD       # 8194
RR = LP                # tokens per sample (l' in [0, 8193])
CO = 64
NQ = 4                 # output quarters
QW = L // NQ           # 2048 output cols per quarter
QB = QW // 64          # 32 q-values per quarter
NQUEUES = 4            # SWDGE gather queues

_CACHE = {}


def _build_nc(debug=False, reps=1, stage='full'):
    import concourse.bass as bass
    import concourse.tile as tile
    from concourse import bacc, mybir

    f32 = mybir.dt.float32
    bf16 = mybir.dt.bfloat16
    i32 = mybir.dt.int32
    i16 = mybir.dt.int16
    Alu = mybir.AluOpType
    Act = mybir.ActivationFunctionType

    nc = bacc.Bacc("TRN2", target_bir_lowering=False, num_swdge_queues=NQUEUES)

    xin = nc.dram_tensor("xin", [NS, C, L], f32, kind="ExternalInput")
    # wb: bf16 [128, 9+192]: rows 0:64 cols 0:9 = woffT [c, j*3+kk];
    # cols 9:201 = wTk [t, k*64+o] duplicated in both partition halves
    # (PE matmul needs lhsT base partition == rhs base partition).
    wb = nc.dram_tensor("wb", [128, 9 + K * CO], bf16, kind="ExternalInput")
    # bias: f32 [64, 2]: col 0 = bout (64 rows); col 1 rows 0:3 = boff
    biasd = nc.dram_tensor("biasd", [CO, 2], f32, kind="ExternalInput")
    out = nc.dram_tensor("out", [NS, CO, L], f32, kind="ExternalOutput")

    if debug:
        xt2p = nc.dram_tensor("xt2p", [NS * RR, 128], bf16, kind="ExternalOutput")
        d_offs = nc.dram_tensor("d_offs", [NS, 3, L], f32, kind="ExternalOutput")
        d_idx = nc.dram_tensor("d_idx", [K, 128, 128], i16, kind="ExternalOutput")
        d_alpha = nc.dram_tensor("d_alpha", [K, 128, 128], bf16, kind="ExternalOutput")
        d_gb = nc.dram_tensor("d_gb", [128, QB, 128], bf16, kind="ExternalOutput")
    else:
        xt2p = nc.dram_tensor("xt2p", [NS * RR, 128], bf16)  # internal scratch
    idxd = nc.dram_tensor("idxd", [K, 16, 1024], i16)  # wrapped idx bounce

    with tile.TileContext(nc) as tc:
      for rep in range(reps):
        with tc.tile_pool(name=f"const{rep}", bufs=1) as constp:
            wb_t = constp.tile([128, 9 + K * CO], bf16)
            nc.sync.dma_start(wb_t[:], wb[:])
            bias_t = constp.tile([CO, 2], f32)
            nc.sync.dma_start(bias_t[:], biasd[:])

            # --- device-generated constants ---
            # identity [64, 64] bf16 for PE transposes
            qi = constp.tile([64, 64], i32)
            nc.gpsimd.iota(qi[:], [[1, 64]], base=0, channel_multiplier=0)
            pv = constp.tile([64, 1], i32)
            nc.gpsimd.iota(pv[:], [[1, 1]], base=0, channel_multiplier=1)
            qi_f = constp.tile([64, 64], f32)
            nc.vector.tensor_copy(qi_f[:], qi[:])
            pv_f = constp.tile([64, 1], f32)
            nc.vector.tensor_copy(pv_f[:], pv[:])
            idn = constp.tile([64, 64], bf16)
            nc.vector.tensor_scalar(idn[:], qi_f[:], pv_f[:], None, op0=Alu.is_equal)
            # base128[p, q] = (p % 64) * 128 + q + 1  (f32)
            bi = constp.tile([128, 128], i32)
            nc.gpsimd.iota(bi[:], [[1, 128]], base=1, channel_multiplier=128)
            base_t = constp.tile([128, 128], f32)
            nc.vector.tensor_copy(base_t[0:64, :], bi[0:64, :])
            nc.vector.tensor_scalar(base_t[64:128, :], bi[64:128, :], 64 * 128,
                                    None, op0=Alu.subtract)
            # rowci[p, q] = RR * (p // 64)  (i32)
            rowci = constp.tile([128, 128], i32)
            nc.vector.memset(rowci[0:64, :], 0)
            nc.vector.memset(rowci[64:128, :], RR)
            # zero row for xt2p boundary tokens
            zrow = constp.tile([1, 128], bf16)
            nc.vector.memset(zrow[:], 0.0)

            alpha_t = [constp.tile([128, 128], bf16, tag=f"alpha{k}",
                                   name=f"al{rep}_{k}") for k in range(K)]
            # wrapped+replicated gather index tiles, all quarters:
            # wkq[k][16g+a, Q*256 + ql*8 + 4n + h] = RR*n + left_k[(16h+a)*128 + Q*32 + ql]
            wkq_t = [constp.tile([128, NQ * 256], i16, tag=f"wk{k}",
                                 name=f"wk{rep}_{k}") for k in range(K)]

            # ---------------- phase 1 ----------------
            with tc.tile_pool(name=f"xp{rep}", bufs=2) as xpp, \
                 tc.tile_pool(name=f"xb{rep}", bufs=2) as xbp, \
                 tc.tile_pool(name=f"offs{rep}", bufs=2) as offsp, \
                 tc.tile_pool(name=f"cpsum{rep}", bufs=1, space="PSUM") as cpsump, \
                 tc.tile_pool(name=f"tpsum{rep}", bufs=2, space="PSUM") as tpsump, \
                 tc.tile_pool(name=f"stage{rep}", bufs=3) as stagep, \
                 tc.tile_pool(name=f"small{rep}", bufs=2) as smallp:

                offs_tiles = []
                for n in range(NS):
                    xp = xpp.tile([C, L], f32, tag="xp")
                    nc.sync.dma_start(xp[:], xin[n])
                    xb = xbp.tile([C, LP], bf16, tag="xb")
                    nc.vector.memset(xb[:, 0:1], 0.0)
                    nc.vector.memset(xb[:, LP - 1 : LP], 0.0)
                    nc.scalar.activation(xb[:, 1 : 1 + L], xp[:], Act.Copy)

                    # offsets conv: off[kk, l'] = sum_c sum_j woff[kk,c,j]*xpad[c, l'+j]
                    offs_n = offsp.tile([3, L], f32, tag="offs")
                    offs_tiles.append(offs_n)
                    for c2 in range(L // 2048):
                        cps = cpsump.tile([3, 2048], f32, tag="cps")
                        for b in range(4):
                            col0 = c2 * 2048 + b * 512
                            for j in range(3):
                                nc.tensor.matmul(
                                    cps[:, b * 512 : (b + 1) * 512],
                                    lhsT=wb_t[0:64, j * 3 : (j + 1) * 3],
                                    rhs=xb[:, j + col0 : j + col0 + 512],
                                    start=(j == 0), stop=(j == 2),
                                )
                        nc.scalar.activation(offs_n[:, c2 * 2048 : (c2 + 1) * 2048],
                                             cps[:], Act.Identity,
                                             bias=bias_t[0:3, 1:2])

                    # transpose xb into xt2p pair-tokens, p-major block
                    # layout: token l' = 1+l0+j*128+p stored at row 1+l0+p*8+j
                    # so each partition writes 8 consecutive 256B rows (2KB).
                    for g in range(8):
                        l0 = g * 1024
                        tpsL = tpsump.tile([128, 512], bf16, tag="tpsL")
                        tpsR = tpsump.tile([128, 512], bf16, tag="tpsR")
                        for j in range(8):
                            nc.tensor.transpose(
                                tpsL[:, j * 64 : (j + 1) * 64],
                                xb[:, 1 + l0 + j * 128 : 1 + l0 + (j + 1) * 128],
                                idn[:],
                            )
                            nc.tensor.transpose(
                                tpsR[:, j * 64 : (j + 1) * 64],
                                xb[:, 2 + l0 + j * 128 : 2 + l0 + (j + 1) * 128],
                                idn[:],
                            )
                        pr = stagep.tile([128, 8, 128], bf16, tag="pr")
                        nc.vector.tensor_copy(
                            pr[:, :, 0:64],
                            tpsL[:].rearrange("p (j c) -> p j c", c=64))
                        nc.vector.tensor_copy(
                            pr[:, :, 64:128],
                            tpsR[:].rearrange("p (j c) -> p j c", c=64))
                        nc.sync.dma_start(
                            xt2p[n * RR + 1 + l0 : n * RR + 1 + l0 + 1024, :]
                            .rearrange("(p j) v -> p j v", p=128),
                            pr[:],
                        )
                        if g == 0:
                            # token l'=0 (row 0) = [xpad[0]=0 | xpad[1]]:
                            # xpad[1] = tpsL block j=0, p=0
                            nc.sync.dma_start(
                                xt2p[n * RR : n * RR + 1, 64:128],
                                pr[0:1, 0:1, 0:64])
                            nc.sync.dma_start(xt2p[n * RR : n * RR + 1, 0:64],
                                              zrow[:1, 0:64])
                    # row 8193 = token l'=8193 = [xpad[8193]=0 | overflow] = 0
                    nc.sync.dma_start(xt2p[n * RR + RR - 1 : n * RR + RR, :],
                                      zrow[:1, :])

                # idx / alpha per tap k, in [p=64n+t, q] layout
                for k in range(K):
                    off128 = smallp.tile([128, 128], f32, tag="off128")
                    for n in range(NS):
                        nc.sync.dma_start(off128[n * 64 : (n + 1) * 64, :],
                                          offs_tiles[n][k : k + 1, :])
                    grid = smallp.tile([128, 128], f32, tag="grid")
                    nc.vector.tensor_tensor(grid[:], off128[:], base_t[:], op=Alu.add)
                    gridc = smallp.tile([128, 128], f32, tag="gridc")
                    nc.vector.tensor_scalar(gridc[:], grid[:], 0.0, float(LP - 1),
                                            op0=Alu.max, op1=Alu.min)
                    # floor(gridc), robust to cast rounding mode:
                    # c = cast(gridc); floor = c - (c > gridc)
                    casti = smallp.tile([128, 128], i32, tag="casti")
                    nc.vector.tensor_copy(casti[:], gridc[:])
                    castf = smallp.tile([128, 128], f32, tag="castf")
                    nc.vector.tensor_copy(castf[:], casti[:])
                    over = smallp.tile([128, 128], f32, tag="over")
                    nc.vector.tensor_tensor(over[:], castf[:], gridc[:], op=Alu.is_gt)
                    leftf = smallp.tile([128, 128], f32, tag="leftf")
                    nc.vector.tensor_tensor(leftf[:], castf[:], over[:],
                                            op=Alu.subtract)
                    alphaf = smallp.tile([128, 128], f32, tag="alphaf")
                    nc.vector.tensor_tensor(alphaf[:], gridc[:], leftf[:],
                                            op=Alu.subtract)
                    nc.vector.tensor_copy(alpha_t[k][:], alphaf[:])
                    # permuted token row (p-major block layout), exact i32:
                    # m = max(left-1, 0); r = 1 + (m & ~1023) + ((m&127)<<3)
                    #     + ((m>>7)&7); r = 0 when left == 0; + RR*n
                    li = smallp.tile([128, 128], i32, tag="li")
                    nc.vector.tensor_copy(li[:], leftf[:])
                    mi = smallp.tile([128, 128], i32, tag="mi")
                    nc.vector.tensor_scalar(mi[:], li[:], 1, 0,
                                            op0=Alu.subtract, op1=Alu.max)
                    l0i = smallp.tile([128, 128], i32, tag="l0i")
                    nc.vector.tensor_scalar(l0i[:], mi[:], -1024, None,
                                            op0=Alu.bitwise_and)
                    pi8 = smallp.tile([128, 128], i32, tag="pi8")
                    nc.vector.tensor_scalar(pi8[:], mi[:], 127, 3,
                                            op0=Alu.bitwise_and,
                                            op1=Alu.arith_shift_left)
                    ji = smallp.tile([128, 128], i32, tag="ji")
                    nc.vector.tensor_scalar(ji[:], mi[:], 7, 7,
                                            op0=Alu.arith_shift_right,
                                            op1=Alu.bitwise_and)
                    r0 = smallp.tile([128, 128], i32, tag="r0")
                    nc.vector.tensor_tensor(r0[:], l0i[:], pi8[:], op=Alu.add)
                    r1 = smallp.tile([128, 128], i32, tag="r1")
                    nc.vector.tensor_tensor(r1[:], r0[:], ji[:], op=Alu.add)
                    pos = smallp.tile([128, 128], i32, tag="pos")
                    nc.vector.tensor_scalar(pos[:], li[:], 1, None, op0=Alu.is_ge)
                    r2 = smallp.tile([128, 128], i32, tag="r2")
                    nc.vector.tensor_scalar(r2[:], r1[:], 1, None, op0=Alu.add)
                    r3 = smallp.tile([128, 128], i32, tag="r3")
                    nc.vector.tensor_tensor(r3[:], r2[:], pos[:], op=Alu.mult)
                    idxi = smallp.tile([128, 128], i32, tag="idxi")
                    nc.vector.tensor_tensor(idxi[:], r3[:], rowci[:], op=Alu.add)
                    idx16 = smallp.tile([128, 128], i16, tag="idx16")
                    nc.vector.tensor_copy(idx16[:], idxi[:])
                    # bounce to DRAM in wrapped layout (one write per (k, n)):
                    # idxd[k][a, 256Q + 8ql + 4n + h] = idx16[64n+16h+a, 32Q+ql]
                    for n in range(NS):
                        dst = bass.AP(
                            tensor=idxd[:].tensor,
                            offset=k * 16 * 1024 + 4 * n,
                            ap=[[1, 4], [1024, 16], [256, NQ], [8, QB]])
                        nc.sync.dma_start(dst, idx16[n * 64 : (n + 1) * 64, :])
                    # read back all quarters with 8x partition replication
                    src = bass.AP(
                        tensor=idxd[:].tensor,
                        offset=k * 16 * 1024,
                        ap=[[0, 8], [1024, 16], [1, 1024]])
                    nc.sync.dma_start(wkq_t[k][:], src)
                    if debug:
                        nc.sync.dma_start(d_idx[k], idx16[:])
                        nc.sync.dma_start(d_alpha[k], alpha_t[k][:])
                if debug:
                    for n in range(NS):
                        nc.sync.dma_start(d_offs[n], offs_tiles[n][:])

            # ---------------- phase 2 ----------------
            if stage == 'p1':
                continue
            gq = 0
            with tc.tile_pool(name=f"gb{rep}", bufs=4) as gbp, \
                 tc.tile_pool(name=f"dd{rep}", bufs=2) as ddp, \
                 tc.tile_pool(name=f"ee{rep}", bufs=2) as eep, \
                 tc.tile_pool(name=f"xd{rep}", bufs=2) as xdp, \
                 tc.tile_pool(name=f"outst{rep}", bufs=2) as outp, \
                 tc.tile_pool(name=f"mpsum{rep}", bufs=1, space="PSUM") as mpsump:
                for Q in range(NQ):
                    ps = [mpsump.tile([CO, QW], f32, tag=f"ps{n}",
                                      name=f"ps{rep}_{n}_{Q}") for n in range(NS)] \
                        if stage != 'nomm' else None
                    for k in range(K):
                        gt = gbp.tile([128, QB, 128], bf16, tag="gb")
                        xt2p_ap = bass.AP(tensor=xt2p[:].tensor, offset=0,
                                          ap=[[128, NS * RR], [1, 128]])
                        nc.gpsimd.dma_gather(
                            gt[:], xt2p_ap,
                            wkq_t[k][:, Q * 256 : (Q + 1) * 256],
                            num_idxs=128 * QB, num_idxs_reg=128 * QB,
                            elem_size=128, elem_step=128, single_packet=False,
                            queue_num=gq % NQUEUES,
                        )
                        gq += 1
                        if debug and Q == 0 and k == 0:
                            nc.sync.dma_start(d_gb[:], gt[:])
                        if stage == 'nomm':
                            continue
                        dt_ = ddp.tile([128, QB, 64], bf16, tag="dd")
                        nc.vector.tensor_tensor(dt_[:], gt[:, :, 64:128],
                                                gt[:, :, 0:64], op=Alu.subtract)
                        et = eep.tile([128, QB, 64], bf16, tag="ee")
                        nc.vector.tensor_tensor(
                            et[:], dt_[:],
                            alpha_t[k][:, Q * QB : (Q + 1) * QB]
                            .to_broadcast([128, QB, 64]),
                            op=Alu.mult)
                        xd = xdp.tile([128, QB, 64], bf16, tag="xd")
                        nc.vector.tensor_tensor(xd[:], gt[:, :, 0:64], et[:],
                                                op=Alu.add)
                        for n in range(NS):
                            lhs = wb_t[n * 64 : (n + 1) * 64,
                                       9 + k * 64 : 9 + (k + 1) * 64]
                            rhs_n = xd[n * 64 : (n + 1) * 64, :, :]
                            for b in range(QW // 512):
                                nc.tensor.matmul(
                                    ps[n][:, b * 512 : (b + 1) * 512],
                                    lhsT=lhs,
                                    rhs=rhs_n[:, b * 8 : (b + 1) * 8, :],
                                    start=(k == 0),
                                    stop=(k == K - 1),
                                )
                    for n in range(NS if stage != 'nomm' else 0):
                        ot = outp.tile([CO, QW], f32, tag=f"ot{n}")
                        nc.scalar.activation(ot[:], ps[n][:], Act.Identity,
                                             bias=bias_t[:, 0:1])
                        nc.sync.dma_start(out[n, :, Q * QW : (Q + 1) * QW], ot[:])

    nc.compile()
    return nc


def _host_tables(w_off, w, b_off, b):
    from concourse import mybir
    bf = mybir.dt.np(mybir.dt.bfloat16)
    woffT = np.ascontiguousarray(
        w_off[[0, 2, 4], :, :].transpose(1, 2, 0).reshape(C, 9)).astype(np.float32)
    wTk = np.ascontiguousarray(
        w.reshape(CO, K, 64).transpose(2, 1, 0).reshape(64, K * CO)).astype(np.float32)
    wb1 = np.concatenate([woffT, wTk], axis=1)
    wb2 = np.concatenate([np.zeros_like(woffT), wTk], axis=1)
    wb = np.concatenate([wb1, wb2], axis=0).astype(bf)
    bias = np.zeros((CO, 2), np.float32)
    bias[:, 0] = np.asarray(b, np.float32)
    bias[0:3, 1] = np.asarray(b_off, np.float32)[[0, 2, 4]]
    return dict(wb=wb, biasd=bias)


def get_nc(debug=False, reps=1, stage='full'):
    key = f"nc_{int(debug)}_{reps}_{stage}"
    if key not in _CACHE:
        _CACHE[key] = _build_nc(debug, reps, stage)
    return _CACHE[key]


def _get_callable(debug=False, reps=1, stage='full'):
    """Fast-dispatch 8-core shard_map program running the NEFF; compiled once."""
    fkey = f"fn_{int(debug)}_{reps}_{stage}"
    if fkey in _CACHE:
        return _CACHE[fkey]
    import jax
    from jax.sharding import Mesh, PartitionSpec, NamedSharding
    from jax.experimental.shard_map import shard_map
    from concourse import bass2jax, mybir

    bass2jax.install_neuronx_cc_hook()
    nc = get_nc(debug, reps, stage)
    partition_name = nc.partition_id_tensor.name if nc.partition_id_tensor else None
    in_names, out_names, out_avals = [], [], []
    for alloc in nc.m.functions[0].allocations:
        if not isinstance(alloc, mybir.MemoryLocationSet):
            continue
        name = alloc.memorylocations[0].name
        if alloc.kind == "ExternalInput":
            if name != partition_name:
                in_names.append(name)
        elif alloc.kind == "ExternalOutput":
            out_names.append(name)
            out_avals.append(jax.core.ShapedArray(
                tuple(alloc.tensor_shape), mybir.dt.np(alloc.dtype)))
    n_params = len(in_names)
    all_in_names = list(in_names) + list(out_names)
    if partition_name is not None:
        all_in_names.append(partition_name)

    def _body(*args):
        operands = list(args)
        if partition_name is not None:
            operands.append(bass2jax.partition_id_tensor())
        outs = bass2jax._bass_exec_p.bind(
            *operands,
            out_avals=tuple(out_avals),
            in_names=tuple(all_in_names),
            out_names=tuple(out_names),
            lowering_input_output_aliases=(),
            sim_require_finite=True,
            sim_require_nnan=True,
            nc=nc,
        )
        return tuple(outs)

    devices = jax.devices()[:NCORES]
    mesh = Mesh(np.asarray(devices), ("core",))
    n_all = n_params + len(out_names)
    jitted = jax.jit(
        shard_map(_body, mesh=mesh,
                  in_specs=(PartitionSpec("core"),) * n_all,
                  out_specs=(PartitionSpec("core"),) * len(out_names),
                  check_rep=False),
        keep_unused=True,
    )
    sh = NamedSharding(mesh, PartitionSpec("core"))
    # Compile the fast-dispatch executable against representative avals.
    dummies = []
    for nm in in_names:
        for alloc in nc.m.functions[0].allocations:
            if (isinstance(alloc, mybir.MemoryLocationSet)
                    and alloc.memorylocations[0].name == nm):
                shp = tuple(alloc.tensor_shape)
                dummies.append(jax.device_put(np.zeros(
                    (NCORES * shp[0], *shp[1:]), mybir.dt.np(alloc.dtype)), sh))
                break
    out_dummies = [jax.device_put(np.zeros(
        (NCORES * av.shape[0], *av.shape[1:]), av.dtype), sh)
        for av in out_avals]
    dummies += out_dummies
    fast = bass2jax.fast_dispatch_compile(lambda: jitted.lower(*dummies).compile())
    _CACHE[fkey] = (fast, in_names, out_names, out_avals, mesh, sh, out_dummies)
    return _CACHE[fkey]


def _device_inputs(x, w_off, b_off, w, b, in_names, sh):
    """Per-call device inputs. x is sliced by the sharding directly; small
    tables are tiled per-core."""
    import jax
    tables = _host_tables(np.asarray(w_off), np.asarray(w),
                          np.asarray(b_off), np.asarray(b))
    x = np.ascontiguousarray(np.asarray(x), dtype=np.float32)
    arrs = []
    for nm in in_names:
        if nm == "xin":
            arrs.append(x)
        else:
            t = tables[nm]
            arrs.append(np.tile(t, (NCORES,) + (1,) * (t.ndim - 1)))
    return [jax.device_put(a, sh) for a in arrs]


def kernel(x, w_off, b_off, w, b):
    fast, in_names, out_names, out_avals, mesh, sh, out_bufs = _get_callable()
    dargs = _device_inputs(x, w_off, b_off, w, b, in_names, sh) + out_bufs
    outs = fast(*dargs)
    oidx = out_names.index("out")
    return np.asarray(outs[oidx])


def timeit(x, w_off, b_off, w, b, iters=30, reps=1, stage='full'):
    import time
    import jax
    fast, in_names, out_names, out_avals, mesh, sh, out_bufs = \
        _get_callable(reps=reps, stage=stage)
    dargs = _device_inputs(x, w_off, b_off, w, b, in_names, sh) + out_bufs
    outs = fast(*dargs)
    jax.block_until_ready(outs)
    t0 = time.perf_counter()
    for _ in range(iters):
        outs = fast(*dargs)
    jax.block_until_ready(outs)
    t1 = time.perf_counter()
    return (t1 - t0) / iters * 1e9


# revision 11
# speedup vs baseline: 2.6142x; 1.0284x over previous
"""DeformableConv1d Trainium2 kernel (bf16 gather rewrite).

Problem: N=16, C_in=64, L=8192, K=3, C_out=64, PAD=1.
Sharding: data-parallel over batch; each of 8 cores handles 2 samples.

Math (validated against the jax reference):
  offsets = conv1d(x, w_off, pad=1) + b_off      (only channels 0,2,4 used)
  grid[l',k] = clip(l'+1 + off_k[l'], 0, 8193)   (padded coords)
  left = floor(grid), alpha = grid - left
  out[n, o, q*64+r] = sum_{k,t} w[o, k*64+t] * xd[n, r, t*128+q, k]
  xd[n, c, l', k] = (1-a)*xp[n, c, left] + a*xp[n, c, left+1]
where l = q*64+r (q in [0,128), r in [0,64)), t in [0,64).

Kernel structure per core (all hot-path data in bf16):
  P1: load x (f32) -> convert bf16 xb; offsets conv on PE (bf16 in, f32 psum);
      PE-transpose xb -> xt2p[2*8194, 128] bf16 DRAM pair-token table
      (token l' = [x_pad[:,l'] | x_pad[:,l'+1]], 256B); idx/alpha tiles
      [128,128] (partition p = 64n+t, free q); wrapped gather-index bounce.
  P2: for each quarter Q and tap k: SWDGE dma_gather (256B tokens,
      round-robin over 4 queues) -> GB[128, 32, 128] bf16; DVE blend
      xd = L + alpha*(R-L); PE matmuls (bf16) acc out += WkT @ xd into
      PSUM [64, 2048] per sample; ACT drain (+bias); DMA out.
"""

import numpy as np

N, C, L, K, PAD = 16, 64, 8192, 3, 1
NS = 2                 # samples per core
NCORES = 8
LP = L + 2 * PAD       # 8194
RR = LP                # tokens per sample (l' in [0, 8193])
CO = 64
NQ = 4                 # output quarters
QW = L // NQ           # 2048 output cols per quarter
QB = QW // 64          # 32 q-values per quarter
NQUEUES = 4            # SWDGE gather queues

_CACHE = {}


def _build_nc(debug=False, reps=1, stage='full'):
    import concourse.bass as bass
    import concourse.tile as tile
    from concourse import bacc, mybir

    f32 = mybir.dt.float32
    bf16 = mybir.dt.bfloat16
    i32 = mybir.dt.int32
    i16 = mybir.dt.int16
    Alu = mybir.AluOpType
    Act = mybir.ActivationFunctionType

    nc = bacc.Bacc("TRN2", target_bir_lowering=False, num_swdge_queues=NQUEUES)

    xin = nc.dram_tensor("xin", [NS, C, L], f32, kind="ExternalInput")
    # wb: bf16 [128, 9+192]: rows 0:64 cols 0:9 = woffT [c, j*3+kk];
    # cols 9:201 = wTk [t, k*64+o] duplicated in both partition halves
    # (PE matmul needs lhsT base partition == rhs base partition).
    wb = nc.dram_tensor("wb", [128, 9 + K * CO], bf16, kind="ExternalInput")
    # bias: f32 [64, 2]: col 0 = bout (64 rows); col 1 rows 0:3 = boff
    biasd = nc.dram_tensor("biasd", [CO, 2], f32, kind="ExternalInput")
    out = nc.dram_tensor("out", [NS, CO, L], f32, kind="ExternalOutput")

    if debug:
        xt2p = nc.dram_tensor("xt2p", [NS * RR, 128], bf16, kind="ExternalOutput")
        d_offs = nc.dram_tensor("d_offs", [NS, 3, L], f32, kind="ExternalOutput")
        d_idx = nc.dram_tensor("d_idx", [K, 128, 128], i16, kind="ExternalOutput")
        d_alpha = nc.dram_tensor("d_alpha", [K, 128, 128], bf16, kind="ExternalOutput")
        d_gb = nc.dram_tensor("d_gb", [128, QB, 128], bf16, kind="ExternalOutput")
    else:
        xt2p = nc.dram_tensor("xt2p", [NS * RR, 128], bf16)  # internal scratch
    idxd = nc.dram_tensor("idxd", [K, 16, 1024], i16)  # wrapped idx bounce

    with tile.TileContext(nc) as tc:
      for rep in range(reps):
        with tc.tile_pool(name=f"const{rep}", bufs=1) as constp:
            wb_t = constp.tile([128, 9 + K * CO], bf16)
            nc.sync.dma_start(wb_t[:], wb[:])
            bias_t = constp.tile([CO, 2], f32)
            nc.sync.dma_start(bias_t[:], biasd[:])

            # --- device-generated constants ---
            # identity [64, 64] bf16 for PE transposes
            qi = constp.tile([64, 64], i32)
            nc.gpsimd.iota(qi[:], [[1, 64]], base=0, channel_multiplier=0)
            pv = constp.tile([64, 1], i32)
            nc.gpsimd.iota(pv[:], [[1, 1]], base=0, channel_multiplier=1)
            qi_f = constp.tile([64, 64], f32)
            nc.vector.tensor_copy(qi_f[:], qi[:])
            pv_f = constp.tile([64, 1], f32)
            nc.vector.tensor_copy(pv_f[:], pv[:])
            idn = constp.tile([64, 64], bf16)
            nc.vector.tensor_scalar(idn[:], qi_f[:], pv_f[:], None, op0=Alu.is_equal)
            # base128[p, q] = (p % 64) * 128 + q + 1  (f32)
            bi = constp.tile([128, 128], i32)
            nc.gpsimd.iota(bi[:], [[1, 128]], base=1, channel_multiplier=128)
            base_t = constp.tile([128, 128], f32)
            nc.vector.tensor_copy(base_t[0:64, :], bi[0:64, :])
            nc.vector.tensor_scalar(base_t[64:128, :], bi[64:128, :], 64 * 128,
                                    None, op0=Alu.subtract)
            # rowci[p, q] = RR * (p // 64)  (i32)
            rowci = constp.tile([128, 128], i32)
            nc.vector.memset(rowci[0:64, :], 0)
            nc.vector.memset(rowci[64:128, :], RR)
            # zero row for xt2p boundary tokens
            zrow = constp.tile([1, 128], bf16)
            nc.vector.memset(zrow[:], 0.0)

            alpha_t = [constp.tile([128, 128], bf16, tag=f"alpha{k}",
                                   name=f"al{rep}_{k}") for k in range(K)]
            # wrapped+replicated gather index tiles, all quarters:
            # wkq[k][16g+a, Q*256 + ql*8 + 4n + h] = RR*n + left_k[(16h+a)*128 + Q*32 + ql]
            wkq_t = [constp.tile([128, NQ * 256], i16, tag=f"wk{k}",
                                 name=f"wk{rep}_{k}") for k in range(K)]

            # ---------------- phase 1 ----------------
            with tc.tile_pool(name=f"xp{rep}", bufs=2) as xpp, \
                 tc.tile_pool(name=f"xb{rep}", bufs=2) as xbp, \
                 tc.tile_pool(name=f"offs{rep}", bufs=2) as offsp, \
                 tc.tile_pool(name=f"cpsum{rep}", bufs=2, space="PSUM") as cpsump, \
                 tc.tile_pool(name=f"tpsum{rep}", bufs=2, space="PSUM") as tpsump, \
                 tc.tile_pool(name=f"stage{rep}", bufs=3) as stagep, \
                 tc.tile_pool(name=f"small{rep}", bufs=2) as smallp:

                offs_tiles = []
                for n in range(NS):
                    xp = xpp.tile([C, L], f32, tag="xp")
                    nc.sync.dma_start(xp[:], xin[n])
                    xb = xbp.tile([C, LP], bf16, tag="xb")
                    nc.vector.memset(xb[:, 0:1], 0.0)
                    nc.vector.memset(xb[:, LP - 1 : LP], 0.0)
                    nc.scalar.activation(xb[:, 1 : 1 + L], xp[:], Act.Copy)

                    # offsets conv: off[kk, l'] = sum_c sum_j woff[kk,c,j]*xpad[c, l'+j]
                    offs_n = offsp.tile([3, L], f32, tag="offs")
                    offs_tiles.append(offs_n)
                    for c2 in range(L // 1024):
                        cps = cpsump.tile([3, 1024], f32, tag="cps")
                        for b in range(2):
                            col0 = c2 * 1024 + b * 512
                            for j in range(3):
                                nc.tensor.matmul(
                                    cps[:, b * 512 : (b + 1) * 512],
                                    lhsT=wb_t[0:64, j * 3 : (j + 1) * 3],
                                    rhs=xb[:, j + col0 : j + col0 + 512],
                                    start=(j == 0), stop=(j == 2),
                                )
                        nc.scalar.activation(offs_n[:, c2 * 1024 : (c2 + 1) * 1024],
                                             cps[:], Act.Identity,
                                             bias=bias_t[0:3, 1:2])

                    # transpose xb into xt2p pair-tokens, p-major block
                    # layout: token l' = 1+l0+j*128+p stored at row 1+l0+p*8+j
                    # so each partition writes 8 consecutive 256B rows (2KB).
                    for g in range(8):
                        l0 = g * 1024
                        tpsL = tpsump.tile([128, 512], bf16, tag="tpsL")
                        tpsR = tpsump.tile([128, 512], bf16, tag="tpsR")
                        for j in range(8):
                            nc.tensor.transpose(
                                tpsL[:, j * 64 : (j + 1) * 64],
                                xb[:, 1 + l0 + j * 128 : 1 + l0 + (j + 1) * 128],
                                idn[:],
                            )
                            nc.tensor.transpose(
                                tpsR[:, j * 64 : (j + 1) * 64],
                                xb[:, 2 + l0 + j * 128 : 2 + l0 + (j + 1) * 128],
                                idn[:],
                            )
                        pr = stagep.tile([128, 8, 128], bf16, tag="pr")
                        nc.vector.tensor_copy(
                            pr[:, :, 0:64],
                            tpsL[:].rearrange("p (j c) -> p j c", c=64))
                        nc.vector.tensor_copy(
                            pr[:, :, 64:128],
                            tpsR[:].rearrange("p (j c) -> p j c", c=64))
                        nc.sync.dma_start(
                            xt2p[n * RR + 1 + l0 : n * RR + 1 + l0 + 1024, :]
                            .rearrange("(p j) v -> p j v", p=128),
                            pr[:],
                        )
                        if g == 0:
                            # token l'=0 (row 0) = [xpad[0]=0 | xpad[1]]:
                            # xpad[1] = tpsL block j=0, p=0
                            nc.sync.dma_start(
                                xt2p[n * RR : n * RR + 1, 64:128],
                                pr[0:1, 0:1, 0:64])
                            nc.sync.dma_start(xt2p[n * RR : n * RR + 1, 0:64],
                                              zrow[:1, 0:64])
                    # row 8193 = token l'=8193 = [xpad[8193]=0 | overflow] = 0
                    nc.sync.dma_start(xt2p[n * RR + RR - 1 : n * RR + RR, :],
                                      zrow[:1, :])

                # idx / alpha per tap k, in [p=64n+t, q] layout
                for k in range(K):
                    off128 = smallp.tile([128, 128], f32, tag="off128")
                    for n in range(NS):
                        nc.sync.dma_start(off128[n * 64 : (n + 1) * 64, :],
                                          offs_tiles[n][k : k + 1, :])
                    grid = smallp.tile([128, 128], f32, tag="grid")
                    nc.vector.tensor_tensor(grid[:], off128[:], base_t[:], op=Alu.add)
                    gridc = smallp.tile([128, 128], f32, tag="gridc")
                    nc.vector.tensor_scalar(gridc[:], grid[:], 0.0, float(LP - 1),
                                            op0=Alu.max, op1=Alu.min)
                    # floor(gridc), robust to cast rounding mode:
                    # c = cast(gridc); floor = c - (c > gridc)
                    casti = smallp.tile([128, 128], i32, tag="casti")
                    nc.vector.tensor_copy(casti[:], gridc[:])
                    castf = smallp.tile([128, 128], f32, tag="castf")
                    nc.vector.tensor_copy(castf[:], casti[:])
                    over = smallp.tile([128, 128], f32, tag="over")
                    nc.vector.tensor_tensor(over[:], castf[:], gridc[:], op=Alu.is_gt)
                    leftf = smallp.tile([128, 128], f32, tag="leftf")
                    nc.vector.tensor_tensor(leftf[:], castf[:], over[:],
                                            op=Alu.subtract)
                    alphaf = smallp.tile([128, 128], f32, tag="alphaf")
                    nc.vector.tensor_tensor(alphaf[:], gridc[:], leftf[:],
                                            op=Alu.subtract)
                    nc.vector.tensor_copy(alpha_t[k][:], alphaf[:])
                    # permuted token row (p-major block layout), exact i32:
                    # m = max(left-1, 0); r = 1 + (m & ~1023) + ((m&127)<<3)
                    #     + ((m>>7)&7); r = 0 when left == 0; + RR*n
                    li = smallp.tile([128, 128], i32, tag="li")
                    nc.vector.tensor_copy(li[:], leftf[:])
                    mi = smallp.tile([128, 128], i32, tag="mi")
                    nc.vector.tensor_scalar(mi[:], li[:], 1, 0,
                                            op0=Alu.subtract, op1=Alu.max)
                    l0i = smallp.tile([128, 128], i32, tag="l0i")
                    nc.vector.tensor_scalar(l0i[:], mi[:], -1024, None,
                                            op0=Alu.bitwise_and)
                    pi8 = smallp.tile([128, 128], i32, tag="pi8")
                    nc.vector.tensor_scalar(pi8[:], mi[:], 127, 3,
                                            op0=Alu.bitwise_and,
                                            op1=Alu.arith_shift_left)
                    ji = smallp.tile([128, 128], i32, tag="ji")
                    nc.vector.tensor_scalar(ji[:], mi[:], 7, 7,
                                            op0=Alu.arith_shift_right,
                                            op1=Alu.bitwise_and)
                    r0 = smallp.tile([128, 128], i32, tag="r0")
                    nc.vector.tensor_tensor(r0[:], l0i[:], pi8[:], op=Alu.add)
                    r1 = smallp.tile([128, 128], i32, tag="r1")
                    nc.vector.tensor_tensor(r1[:], r0[:], ji[:], op=Alu.add)
                    pos = smallp.tile([128, 128], i32, tag="pos")
                    nc.vector.tensor_scalar(pos[:], li[:], 1, None, op0=Alu.is_ge)
                    r2 = smallp.tile([128, 128], i32, tag="r2")
                    nc.vector.tensor_scalar(r2[:], r1[:], 1, None, op0=Alu.add)
                    r3 = smallp.tile([128, 128], i32, tag="r3")
                    nc.vector.tensor_tensor(r3[:], r2[:], pos[:], op=Alu.mult)
                    idxi = smallp.tile([128, 128], i32, tag="idxi")
                    nc.vector.tensor_tensor(idxi[:], r3[:], rowci[:], op=Alu.add)
                    idx16 = smallp.tile([128, 128], i16, tag="idx16")
                    nc.vector.tensor_copy(idx16[:], idxi[:])
                    # bounce to DRAM in wrapped layout (one write per (k, n)):
                    # idxd[k][a, 256Q + 8ql + 4n + h] = idx16[64n+16h+a, 32Q+ql]
                    for n in range(NS):
                        dst = bass.AP(
                            tensor=idxd[:].tensor,
                            offset=k * 16 * 1024 + 4 * n,
                            ap=[[1, 4], [1024, 16], [256, NQ], [8, QB]])
                        nc.sync.dma_start(dst, idx16[n * 64 : (n + 1) * 64, :])
                    # read back all quarters with 8x partition replication
                    src = bass.AP(
                        tensor=idxd[:].tensor,
                        offset=k * 16 * 1024,
                        ap=[[0, 8], [1024, 16], [1, 1024]])
                    nc.sync.dma_start(wkq_t[k][:], src)
                    if debug:
                        nc.sync.dma_start(d_idx[k], idx16[:])
                        nc.sync.dma_start(d_alpha[k], alpha_t[k][:])
                if debug:
                    for n in range(NS):
                        nc.sync.dma_start(d_offs[n], offs_tiles[n][:])

            # ---------------- phase 2 ----------------
            if stage == 'p1':
                continue
            gq = 0
            with tc.tile_pool(name=f"gb{rep}", bufs=6) as gbp, \
                 tc.tile_pool(name=f"dd{rep}", bufs=2) as ddp, \
                 tc.tile_pool(name=f"ee{rep}", bufs=2) as eep, \
                 tc.tile_pool(name=f"xd{rep}", bufs=2) as xdp, \
                 tc.tile_pool(name=f"outst{rep}", bufs=2) as outp, \
                 tc.tile_pool(name=f"mpsum{rep}", bufs=1, space="PSUM") as mpsump:
                for Q in range(NQ):
                    ps = [mpsump.tile([CO, QW], f32, tag=f"ps{n}",
                                      name=f"ps{rep}_{n}_{Q}") for n in range(NS)] \
                        if stage != 'nomm' else None
                    for k in range(K):
                        gt = gbp.tile([128, QB, 128], bf16, tag="gb")
                        xt2p_ap = bass.AP(tensor=xt2p[:].tensor, offset=0,
                                          ap=[[128, NS * RR], [1, 128]])
                        for h2 in range(2):
                            nc.gpsimd.dma_gather(
                                gt[:, 16 * h2 : 16 * (h2 + 1), :], xt2p_ap,
                                wkq_t[k][:, Q * 256 + 128 * h2
                                          : Q * 256 + 128 * (h2 + 1)],
                                num_idxs=2048, num_idxs_reg=2048,
                                elem_size=128, elem_step=128,
                                single_packet=False,
                                queue_num=gq % NQUEUES,
                            )
                            gq += 1
                        if debug and Q == 0 and k == 0:
                            nc.sync.dma_start(d_gb[:], gt[:])
                        if stage == 'nomm':
                            continue
                        dt_ = ddp.tile([128, QB, 64], bf16, tag="dd")
                        nc.vector.tensor_tensor(dt_[:], gt[:, :, 64:128],
                                                gt[:, :, 0:64], op=Alu.subtract)
                        et = eep.tile([128, QB, 64], bf16, tag="ee")
                        nc.vector.tensor_tensor(
                            et[:], dt_[:],
                            alpha_t[k][:, Q * QB : (Q + 1) * QB]
                            .to_broadcast([128, QB, 64]),
                            op=Alu.mult)
                        xd = xdp.tile([128, QB, 64], bf16, tag="xd")
                        nc.vector.tensor_tensor(xd[:], gt[:, :, 0:64], et[:],
                                                op=Alu.add)
                        for n in range(NS):
                            lhs = wb_t[n * 64 : (n + 1) * 64,
                                       9 + k * 64 : 9 + (k + 1) * 64]
                            rhs_n = xd[n * 64 : (n + 1) * 64, :, :]
                            for b in range(QW // 512):
                                nc.tensor.matmul(
                                    ps[n][:, b * 512 : (b + 1) * 512],
                                    lhsT=lhs,
                                    rhs=rhs_n[:, b * 8 : (b + 1) * 8, :],
                                    start=(k == 0),
                                    stop=(k == K - 1),
                                )
                    for n in range(NS if stage != 'nomm' else 0):
                        ot = outp.tile([CO, QW], f32, tag=f"ot{n}")
                        nc.scalar.activation(ot[:], ps[n][:], Act.Identity,
                                             bias=bias_t[:, 0:1])
                        nc.sync.dma_start(out[n, :, Q * QW : (Q + 1) * QW], ot[:])

    nc.compile()
    return nc


def _host_tables(w_off, w, b_off, b):
    from concourse import mybir
    bf = mybir.dt.np(mybir.dt.bfloat16)
    woffT = np.ascontiguousarray(
        w_off[[0, 2, 4], :, :].transpose(1, 2, 0).reshape(C, 9)).astype(np.float32)
    wTk = np.ascontiguousarray(
        w.reshape(CO, K, 64).transpose(2, 1, 0).reshape(64, K * CO)).astype(np.float32)
    wb1 = np.concatenate([woffT, wTk], axis=1)
    wb2 = np.concatenate([np.zeros_like(woffT), wTk], axis=1)
    wb = np.concatenate([wb1, wb2], axis=0).astype(bf)
    bias = np.zeros((CO, 2), np.float32)
    bias[:, 0] = np.asarray(b, np.float32)
    bias[0:3, 1] = np.asarray(b_off, np.float32)[[0, 2, 4]]
    return dict(wb=wb, biasd=bias)


def get_nc(debug=False, reps=1, stage='full'):
    key = f"nc_{int(debug)}_{reps}_{stage}"
    if key not in _CACHE:
        _CACHE[key] = _build_nc(debug, reps, stage)
    return _CACHE[key]


def _get_callable(debug=False, reps=1, stage='full'):
    """Fast-dispatch 8-core shard_map program running the NEFF; compiled once."""
    fkey = f"fn_{int(debug)}_{reps}_{stage}"
    if fkey in _CACHE:
        return _CACHE[fkey]
    import jax
    from jax.sharding import Mesh, PartitionSpec, NamedSharding
    from jax.experimental.shard_map import shard_map
    from concourse import bass2jax, mybir

    bass2jax.install_neuronx_cc_hook()
    nc = get_nc(debug, reps, stage)
    partition_name = nc.partition_id_tensor.name if nc.partition_id_tensor else None
    in_names, out_names, out_avals = [], [], []
    for alloc in nc.m.functions[0].allocations:
        if not isinstance(alloc, mybir.MemoryLocationSet):
            continue
        name = alloc.memorylocations[0].name
        if alloc.kind == "ExternalInput":
            if name != partition_name:
                in_names.append(name)
        elif alloc.kind == "ExternalOutput":
            out_names.append(name)
            out_avals.append(jax.core.ShapedArray(
                tuple(alloc.tensor_shape), mybir.dt.np(alloc.dtype)))
    n_params = len(in_names)
    all_in_names = list(in_names) + list(out_names)
    if partition_name is not None:
        all_in_names.append(partition_name)

    def _body(*args):
        operands = list(args)
        if partition_name is not None:
            operands.append(bass2jax.partition_id_tensor())
        outs = bass2jax._bass_exec_p.bind(
            *operands,
            out_avals=tuple(out_avals),
            in_names=tuple(all_in_names),
            out_names=tuple(out_names),
            lowering_input_output_aliases=(),
            sim_require_finite=True,
            sim_require_nnan=True,
            nc=nc,
        )
        return tuple(outs)

    devices = jax.devices()[:NCORES]
    mesh = Mesh(np.asarray(devices), ("core",))
    n_all = n_params + len(out_names)
    jitted = jax.jit(
        shard_map(_body, mesh=mesh,
                  in_specs=(PartitionSpec("core"),) * n_all,
                  out_specs=(PartitionSpec("core"),) * len(out_names),
                  check_rep=False),
        keep_unused=True,
    )
    sh = NamedSharding(mesh, PartitionSpec("core"))
    # Compile the fast-dispatch executable against representative avals.
    dummies = []
    for nm in in_names:
        for alloc in nc.m.functions[0].allocations:
            if (isinstance(alloc, mybir.MemoryLocationSet)
                    and alloc.memorylocations[0].name == nm):
                shp = tuple(alloc.tensor_shape)
                dummies.append(jax.device_put(np.zeros(
                    (NCORES * shp[0], *shp[1:]), mybir.dt.np(alloc.dtype)), sh))
                break
    out_dummies = [jax.device_put(np.zeros(
        (NCORES * av.shape[0], *av.shape[1:]), av.dtype), sh)
        for av in out_avals]
    dummies += out_dummies
    fast = bass2jax.fast_dispatch_compile(lambda: jitted.lower(*dummies).compile())
    _CACHE[fkey] = (fast, in_names, out_names, out_avals, mesh, sh, out_dummies)
    return _CACHE[fkey]


def _device_inputs(x, w_off, b_off, w, b, in_names, sh):
    """Per-call device inputs. x is sliced by the sharding directly; small
    tables are tiled per-core."""
    import jax
    tables = _host_tables(np.asarray(w_off), np.asarray(w),
                          np.asarray(b_off), np.asarray(b))
    x = np.ascontiguousarray(np.asarray(x), dtype=np.float32)
    arrs = []
    for nm in in_names:
        if nm == "xin":
            arrs.append(x)
        else:
            t = tables[nm]
            arrs.append(np.tile(t, (NCORES,) + (1,) * (t.ndim - 1)))
    return [jax.device_put(a, sh) for a in arrs]


def kernel(x, w_off, b_off, w, b):
    fast, in_names, out_names, out_avals, mesh, sh, out_bufs = _get_callable()
    dargs = _device_inputs(x, w_off, b_off, w, b, in_names, sh) + out_bufs
    outs = fast(*dargs)
    oidx = out_names.index("out")
    return np.asarray(outs[oidx])


def timeit(x, w_off, b_off, w, b, iters=30, reps=1, stage='full'):
    import time
    import jax
    fast, in_names, out_names, out_avals, mesh, sh, out_bufs = \
        _get_callable(reps=reps, stage=stage)
    dargs = _device_inputs(x, w_off, b_off, w, b, in_names, sh) + out_bufs
    outs = fast(*dargs)
    jax.block_until_ready(outs)
    t0 = time.perf_counter()
    for _ in range(iters):
        outs = fast(*dargs)
    jax.block_until_ready(outs)
    t1 = time.perf_counter()
    return (t1 - t0) / iters * 1e9
